# revision 24
# baseline (speedup 1.0000x reference)
"""Trainium2 Bass kernel for nn_DirectionalContrastiveLoss (8-core SPMD).

Strategy (per spec sharding hint): shard the anchor/row dimension across the 8
cores, replicate the host-assembled memory bank, compute each core's score
block locally, and combine masked sums / counts on the host.

V2 layout (vs the earlier selector-based version):
- The label mask is folded INTO the matmul: the contraction gains a third
  K=32 one-hot pass (sqrt(P)*onehot(mem_label_row) vs -sqrt(P)*onehot(
  anchor_label_col), P=1000) so killed columns leave the PE at score-P and
  vanish in the exp.  No kill-range re-reads, no selector matrix.
- Fills are exact 1024-column PSUM tiles (2 banks x 4 slots) so the DVE max
  and ACT exp passes pay the per-instruction overhead only 8x per chain.
- pos (positive logits) and the pos-masks are host-precomputed inputs.
- Per-chain flash combine: GPSIMD min-tree for the running max, one fused
  DVE tensor_tensor_reduce for sum(sparts*edel).
"""
import numpy as np
import ml_dtypes

import bass_rust
import concourse.bass as bass
import concourse.tile as tile
from concourse import mybir
from concourse.bass_utils import run_bass_kernel_spmd
from concourse.vector_clock import ScopedClock

BF16 = ml_dtypes.bfloat16
N_CORES = 8
TEMP = 0.1
POS_THRESH = 0.7
EPS = 1e-8
N = 8000          # anchors (== memory slots)
C = 256           # feature channels
NLAB = 21         # pseudo-label values 0..20
RPC = 1024        # rows per core per direction (padded)
NT = RPC // 128   # row tiles per direction
FILL = 1024       # PSUM fill width (2 banks of fp32)
MM_CHUNK = 512    # matmul free-dim chunk (1 PSUM bank)
SQP = 31.625      # sqrt(penalty); exactly representable in bf16; P ~ 1000

FILLS = [(s, min(FILL, N - s)) for s in range(0, N, FILL)]
NF = len(FILLS)

LAST_RESULTS = None  # BassKernelResults of the most recent kernel() call

# ---------------------------------------------------------------------------
# walrus in this toolchain rejects >1 sync wait per instruction; spread the
# TileContext tail-drain waits over single-wait sync NOPs.
_N_SPILL_NOPS = 64


def _patched_drain_and_barrier(self, tick_clock, wait_clock):
    nops = [self.nc.sync.nop(nofuse=True, hint=f"drainwait{i}")
            for i in range(_N_SPILL_NOPS)]
    drain_inst = self.nc.sync.drain()
    wait_clock.add_sem_waits(drain_inst.ins,
                             ScopedClock({None: tick_clock.global_clock}))
    si = drain_inst.ins.sync_info
    waits = list(si.on_wait) if si is not None else []
    if waits:
        assert len(waits) <= _N_SPILL_NOPS
        for i, w in enumerate(waits):
            nops[i].ins.sync_info = bass_rust.SyncInfo(on_wait=[w], on_update=[])
        drain_inst.ins.sync_info = bass_rust.SyncInfo(
            on_wait=[], on_update=list(si.on_update))
    self.nc.all_engine_barrier()
    popped = self.nc._tile_sem_poison_stack.pop()
    assert popped is self._sem_poison
    self.nc.clear_and_free_semaphores(list(self.sems.allocated().values()))


tile.TileContext._drain_and_barrier = _patched_drain_and_barrier

# Same walrus limitation for regular scheduled instructions: split any
# multi-wait instruction into single-wait same-engine NOPs + the instruction
# keeping its last wait (sequential waits on one engine are equivalent).
_orig_lower_ordered = tile.TileContext._lower_ordered_insts


def _split_multiwait_lower(self, ordered):
    for bb, insts in ordered.items():
        out = []
        for inst in insts:
            si = inst.sync_info
            waits = list(si.on_wait) if si is not None else []
            if len(waits) > 1:
                for w in waits[:-1]:
                    out.append(mybir.InstNoOp(
                        name=self.nc.get_next_instruction_name(),
                        sync_info=mybir.SyncInfo(on_wait=[w], on_update=[]),
                        engine=inst.engine,
                        bass_nofuse=True,
                        text_hint="waitsplit",
                    ))
                inst.sync_info = mybir.SyncInfo(
                    on_wait=[waits[-1]], on_update=list(si.on_update))
            out.append(inst)
        ordered[bb] = out
    return _orig_lower_ordered(self, ordered)


tile.TileContext._lower_ordered_insts = _split_multiwait_lower


# ---------------------------------------------------------------------------
def _build_program(emit):
    """Build the SPMD Bass program (shared by all 8 cores).

    emit: set of (d, t, col_start) 512-col chunks that need the penalty
    matmul pass (a label group killed by some row of tile t intersects it).
    """
    nc = bass.Bass("TRN2", target_bir_lowering=False, debug=False,
                   num_devices=N_CORES)
    f32, bf16 = mybir.dt.float32, mybir.dt.bfloat16
    AX = mybir.AxisListType.X
    OP = mybir.AluOpType
    ACT = mybir.ActivationFunctionType

    d_bank = [nc.dram_tensor(f"bank{d}", [2, 128, N], bf16,
                             kind="ExternalInput").ap() for d in range(2)]
    d_fT = [nc.dram_tensor(f"f{d}T", [2, 128, RPC], bf16,
                           kind="ExternalInput").ap() for d in range(2)]
    d_colpen = [nc.dram_tensor(f"colpen{d}", [32, N], bf16,
                               kind="ExternalInput").ap() for d in range(2)]
    d_rowpen = nc.dram_tensor("rowpen", [32, RPC], bf16,
                              kind="ExternalInput").ap()
    d_negpos = nc.dram_tensor("negpos", [128, NT], f32,
                              kind="ExternalInput").ap()
    d_pm = [nc.dram_tensor(f"pm{d}", [128, NT], f32,
                           kind="ExternalInput").ap() for d in range(2)]
    d_out = nc.dram_tensor("partials", [128, 2], f32, kind="ExternalOutput").ap()

    with tile.TileContext(nc) as tc:
        import contextlib
        with contextlib.ExitStack() as ctx:
            singles = ctx.enter_context(tc.tile_pool(name="singles", bufs=1))
            psum = ctx.enter_context(tc.tile_pool(name="psum", bufs=4, space="PSUM"))
            stats = ctx.enter_context(tc.tile_pool(name="stats", bufs=12))
            scratch = ctx.enter_context(tc.tile_pool(name="scratch", bufs=3))

            # ---- resident inputs ----
            bank = [[singles.tile([128, N], bf16, tag=f"bank{d}k{k}", name=f"bank{d}k{k}")
                     for k in range(2)] for d in range(2)]
            fT = [[singles.tile([128, RPC], bf16, tag=f"fT{d}k{k}", name=f"fT{d}k{k}")
                   for k in range(2)] for d in range(2)]
            colpen = [singles.tile([32, N], bf16, tag=f"colpen{d}", name=f"colpen{d}")
                      for d in range(2)]
            rowpen = singles.tile([32, RPC], bf16, tag="rowpen", name="rowpen")
            negpos = singles.tile([128, NT], f32, tag="negpos", name="negpos")
            pm = [singles.tile([128, NT], f32, tag=f"pm{d}", name=f"pm{d}") for d in range(2)]
            mcol = [singles.tile([128, NT], f32, tag=f"mcol{d}", name=f"mcol{d}") for d in range(2)]
            scol = [singles.tile([128, NT], f32, tag=f"scol{d}", name=f"scol{d}") for d in range(2)]
            numc = [singles.tile([128, NT], f32, tag=f"numc{d}", name=f"numc{d}") for d in range(2)]

            # Each dma_start is a ~650ns DIRECT2D instruction on the serial
            # Sync queue, so the descriptor COUNT (not transfer size) gates
            # the data feed.  Keep only chain 0's first-fill pieces plus a
            # couple of follow-ups on the sync queue, and push the bulk
            # transfers onto the idle GpSimd queue (SWDGE).
            nc.sync.dma_start(out=fT[0][0], in_=d_fT[0][0])
            nc.sync.dma_start(out=bank[0][0][:, 0:1024], in_=d_bank[0][0][:, 0:1024])
            nc.sync.dma_start(out=fT[0][1], in_=d_fT[0][1])
            nc.sync.dma_start(out=bank[0][1][:, 0:1024], in_=d_bank[0][1][:, 0:1024])
            nc.sync.dma_start(out=rowpen, in_=d_rowpen)
            nc.sync.dma_start(out=colpen[0][:, 0:1024], in_=d_colpen[0][:, 0:1024])
            nc.sync.dma_start(out=bank[0][0][:, 1024:3072], in_=d_bank[0][0][:, 1024:3072])
            nc.sync.dma_start(out=bank[0][1][:, 1024:3072], in_=d_bank[0][1][:, 1024:3072])
            nc.sync.dma_start(out=colpen[0][:, 1024:N], in_=d_colpen[0][:, 1024:N])
            nc.sync.dma_start(out=negpos, in_=d_negpos)
            for cst in (3072, 5120, 7168):
                ce = min(cst + 2048, N)
                for k in range(2):
                    nc.sync.dma_start(out=bank[0][k][:, cst:ce],
                                      in_=d_bank[0][k][:, cst:ce])
            for d in range(2):
                nc.sync.dma_start(out=pm[d], in_=d_pm[d])
            for k in range(2):
                nc.sync.dma_start(out=fT[1][k], in_=d_fT[1][k])
            nc.sync.dma_start(out=colpen[1], in_=d_colpen[1])
            for cst in (0, 2048, 4096, 6144):
                ce = min(cst + 2048, N)
                for k in range(2):
                    nc.sync.dma_start(out=bank[1][k][:, cst:ce],
                                      in_=d_bank[1][k][:, cst:ce])

            # Preload the Exp/Ln activation table off the critical path: a
            # dummy exp on a memset tile runs as soon as the engines start.
            warm = singles.tile([128, 1], f32, tag="warm", name="warm")
            nc.gpsimd.memset(warm, 0.0)
            warm2 = singles.tile([128, 1], f32, tag="warm2", name="warm2")
            nc.scalar.activation(out=warm2, in_=warm, func=ACT.Exp,
                                 bias=warm[:, 0:1], scale=1.0)

            # Warm the PE (HAM un-throttles after ~3.4us of sustained MMs):
            # dummy matmuls on never-written SBUF garbage, into the first
            # PSUM pool slot; chain 0's fill 0 overwrites it (start=True).
            wsrc = singles.tile([128, 640], bf16, tag="wsrc", name="wsrc")
            nc.gpsimd.memset(wsrc, 0.0)
            wps = psum.tile([128, FILL], f32, tag="ps", name="wps")
            for i in range(22):
                nc.tensor.matmul(wps[:, 0:512], wsrc[:, 0:128],
                                 wsrc[:, 128:640], start=True, stop=True)

            # ---- main loop: 16 chains (2 directions x 8 row tiles) ----
            def chain(d, t):
                    lhs = [fT[d][k][:, t * 128:(t + 1) * 128] for k in range(2)]
                    rp = rowpen[:, t * 128:(t + 1) * 128]
                    # slot NF carries the pos column: negm gets -pos (so the
                    # min-reduce yields -max(fill maxes, pos)) and sparts gets
                    # 1.0 (so sum(sparts*edel) includes exp(pos - m-hat)).
                    negm = stats.tile([128, NF + 1], f32, tag="negm", name="negm")
                    sparts = stats.tile([128, NF + 1], f32, tag="sparts", name="sparts")
                    nc.gpsimd.tensor_copy(out=negm[:, NF:NF + 1],
                                          in_=negpos[:, t:t + 1])
                    nc.gpsimd.memset(sparts[:, NF:NF + 1], 1.0)
                    for fi, (cst, w) in enumerate(FILLS):
                        ps = psum.tile([128, FILL], f32, tag="ps", name="ps")
                        chunks = []
                        off = 0
                        while off < w:
                            cw = min(MM_CHUNK, w - off)
                            chunks.append((off, cw, (d, t, cst + off) in emit))
                            off += cw
                        # k-major order: one LDWEIGHTS per contraction pass
                        # per fill instead of one per chunk.
                        for k in range(2):
                            for (off, cw, pen) in chunks:
                                nc.tensor.matmul(
                                    ps[:, off:off + cw], lhs[k],
                                    bank[d][k][:, cst + off:cst + off + cw],
                                    start=(k == 0),
                                    stop=(k == 1 and not pen))
                        for (off, cw, pen) in chunks:
                            if pen:
                                nc.tensor.matmul(
                                    ps[:, off:off + cw], rp,
                                    colpen[d][:, cst + off:cst + off + cw],
                                    start=False, stop=True)
                        nc.vector.reduce_max(out=negm[:, fi:fi + 1], in_=ps[:, :w],
                                             axis=AX, negate=True)
                        eo = scratch.tile([128, FILL], bf16, tag="eo", name="eo")
                        nc.scalar.activation(
                            out=eo[:, :w], in_=ps[:, :w], func=ACT.Exp,
                            bias=negm[:, fi:fi + 1], scale=1.0,
                            accum_out=sparts[:, fi:fi + 1])
                        yield
                    # flash combine: m-hat = max(fill maxes, pos), via one DVE
                    # min-reduce in the negated domain (slot NF holds -pos).
                    nc.vector.tensor_reduce(out=mcol[d][:, t:t + 1], in_=negm,
                                            axis=AX, op=OP.min)
                    # edel_f = exp(m_f - m-hat); slot NF = exp(pos - m-hat)
                    # which is exactly the logit numerator.
                    edel = stats.tile([128, NF + 1], f32, tag="edel", name="edel")
                    nc.scalar.activation(out=edel, in_=negm, func=ACT.Exp,
                                         bias=mcol[d][:, t:t + 1], scale=-1.0)
                    nc.gpsimd.tensor_copy(out=numc[d][:, t:t + 1],
                                          in_=edel[:, NF:NF + 1])
                    # scol = sum_f sparts_f*edel_f + exp(pos - m-hat): the
                    # full softmax denominator including the pos column, via
                    # one fused multiply+accumulate DVE op.
                    sprod = stats.tile([128, NF + 1], f32, tag="sprod", name="sprod")
                    nc.vector.scalar_tensor_tensor(
                        out=sprod, in0=sparts, scalar=1.0, in1=edel,
                        op0=OP.mult, op1=OP.mult,
                        accum_out=scol[d][:, t:t + 1])

            # ---- final math for one direction, batched over row tiles ----
            outt = singles.tile([128, 2], f32, tag="outt", name="outt")

            def final(d):
                den = stats.tile([128, NT], f32, tag="den", name="den")
                nc.vector.tensor_single_scalar(out=den, in_=scol[d], scalar=EPS,
                                               op=OP.add)
                rec = stats.tile([128, NT], f32, tag="rec", name="rec")
                nc.vector.reciprocal(out=rec, in_=den)
                lg = stats.tile([128, NT], f32, tag="lg", name="lg")
                nc.vector.tensor_tensor(out=lg, in0=numc[d], in1=rec, op=OP.mult)
                lga = stats.tile([128, NT], f32, tag="lga", name="lga")
                nc.vector.tensor_single_scalar(out=lga, in_=lg, scalar=EPS, op=OP.add)
                ll = stats.tile([128, NT], f32, tag="ll", name="ll")
                nc.scalar.activation(out=ll, in_=lga, func=ACT.Ln)
                lm = stats.tile([128, NT], f32, tag="lm", name="lm")
                nc.vector.tensor_tensor(out=lm, in0=ll, in1=pm[d], op=OP.mult)
                nc.vector.reduce_sum(out=outt[:, d:d + 1], in_=lm, axis=AX)

            # Drive: interleave the first two chains fill-by-fill so chain 0's
            # DMA-paced head is hidden behind chain 1's compute; each
            # direction's final math is emitted as soon as its chains finish.
            for d in range(2):
                if d == 0:
                    alive = [chain(0, 0), chain(0, 1), chain(0, 2)]
                    while alive:
                        for g in list(alive):
                            try:
                                next(g)
                            except StopIteration:
                                alive.remove(g)
                    rest = range(3, NT)
                else:
                    rest = range(NT)
                for t in rest:
                    for _ in chain(d, t):
                        pass
                final(d)
            nc.sync.dma_start(out=d_out, in_=outt)

    return nc


# ---------------------------------------------------------------------------
def kernel(output_feat1, output_feat2, pseudo_label1, pseudo_label2,
           pseudo_logits1, pseudo_logits2, output_ul1, output_ul2,
           selected_idx1, selected_idx2):
    f1 = np.ascontiguousarray(np.asarray(output_feat1, dtype=np.float32))
    f2 = np.ascontiguousarray(np.asarray(output_feat2, dtype=np.float32))
    pl1 = np.asarray(pseudo_label1).astype(np.int64)
    pl2 = np.asarray(pseudo_label2).astype(np.int64)
    pg1 = np.asarray(pseudo_logits1, dtype=np.float32)
    pg2 = np.asarray(pseudo_logits2, dtype=np.float32)
    ul1 = np.asarray(output_ul1, dtype=np.float32)
    ul2 = np.asarray(output_ul2, dtype=np.float32)
    idx1 = np.asarray(selected_idx1).astype(np.int64)
    idx2 = np.asarray(selected_idx2).astype(np.int64)

    b, c, h, w = ul1.shape
    ul1f = ul1.transpose(0, 2, 3, 1).reshape(-1, c)
    ul2f = ul2.transpose(0, 2, 3, 1).reshape(-1, c)
    bank_vals = np.concatenate([ul1f[idx1], ul2f[idx2]], axis=0)   # [N, C]
    ml = np.concatenate([pl1[idx1], pl2[idx2]], axis=0)            # [N]

    # --- column layout per direction (transposed-bug mask: col j kills rows
    # whose memory label equals pl_d[j]); columns sorted by pl_d.
    banks, colpens, colranges = [], [], []
    for pl in (pl1, pl2):
        order = np.argsort(pl, kind="stable")
        sizes = np.bincount(pl, minlength=NLAB)
        starts = np.concatenate([[0], np.cumsum(sizes)])
        colranges.append([(int(starts[v]), int(starts[v + 1])) for v in range(NLAB)])
        bT = np.ascontiguousarray(bank_vals[order].T.astype(BF16))  # [C, N]
        banks.append(bT.reshape(2, 128, N))
        cp = np.zeros((32, N), dtype=np.float32)
        cp[pl[order], np.arange(N)] = -SQP
        colpens.append(cp.astype(BF16))

    # --- row layout: label-sorted (by memory label) with fixed per-core quotas
    nv = np.bincount(ml, minlength=NLAB)
    qv = (nv + N_CORES - 1) // N_CORES
    assert qv.sum() <= RPC
    row_segs = []
    p = 0
    for v in range(NLAB):
        if qv[v] > 0:
            row_segs.append((p, p + int(qv[v]), v))
            p += int(qv[v])

    rows_sorted = np.argsort(ml, kind="stable")
    starts_ml = np.concatenate([[0], np.cumsum(nv)])
    perms = np.full((N_CORES, RPC), -1, dtype=np.int64)
    for v in range(NLAB):
        seg = next(s for s in row_segs if s[2] == v)
        rows_v = rows_sorted[starts_ml[v]:starts_ml[v + 1]]
        for core in range(N_CORES):
            chunk = rows_v[core * qv[v]:(core + 1) * qv[v]]
            perms[core, seg[0]:seg[0] + len(chunk)] = chunk

    # rowpen: [32, RPC] one-hot of each padded row's memory label, sqrt(P).
    rowpen = np.zeros((32, RPC), dtype=np.float32)
    rowlab = np.full(RPC, -1, dtype=np.int64)
    for (s0, s1, v) in row_segs:
        rowlab[s0:s1] = v
        rowpen[v, s0:s1] = SQP
    rowpen = rowpen.astype(BF16)

    # penalty-chunk emit set: (d, t, chunk_start) where a label group killed
    # by some row of tile t intersects the 512-col chunk.
    emit = set()
    for d in range(2):
        for t in range(NT):
            labs = set(rowlab[t * 128:(t + 1) * 128].tolist()) - {-1}
            for (cst, wf) in FILLS:
                off = 0
                while off < wf:
                    cw = min(MM_CHUNK, wf - off)
                    c0, c1 = cst + off, cst + off + cw
                    for v in labs:
                        g0, g1 = colranges[d][v]
                        if g0 < c1 and c0 < g1:
                            emit.add((d, t, c0))
                            break
                    off += cw

    # --- per-core input maps
    def gather_rows(x, perm):
        out = np.zeros((RPC,) + x.shape[1:], dtype=x.dtype)
        msk = perm >= 0
        out[msk] = x[perm[msk]]
        return out

    # host-side row stats: pos (identical for both directions), pos masks
    pos_full = (f1 * f2).sum(axis=1) * (1.0 / TEMP)                # [N]
    pm_full = [((pg2 > POS_THRESH) & (pg1 < pg2)).astype(np.float32),
               ((pg1 > POS_THRESH) & (pg2 < pg1)).astype(np.float32)]
    counts = [float(pm_full[0].sum()), float(pm_full[1].sum())]

    def tcol(x):  # [RPC] -> [128, NT] (tile-major columns)
        return np.ascontiguousarray(x.reshape(NT, 128).T)

    in_maps = []
    for core in range(N_CORES):
        perm = perms[core]
        fc = [gather_rows(f1, perm), gather_rows(f2, perm)]
        posc = gather_rows(pos_full, perm)
        m = {"rowpen": rowpen, "negpos": tcol(-posc)}
        for d in range(2):
            m[f"bank{d}"] = banks[d]
            m[f"colpen{d}"] = colpens[d]
            fT = np.ascontiguousarray((fc[d].T * (1.0 / TEMP)).astype(BF16))
            m[f"f{d}T"] = fT.reshape(2, 128, RPC)
            m[f"pm{d}"] = tcol(gather_rows(pm_full[d], perm))
        in_maps.append(m)

    nc = _build_program(emit)
    res = run_bass_kernel_spmd(nc, in_maps, list(range(N_CORES)))
    global LAST_RESULTS
    LAST_RESULTS = res

    tot = np.zeros(2, dtype=np.float64)
    for core in range(N_CORES):
        tot += res.results[core]["partials"].astype(np.float64).sum(axis=0)
    loss1 = -tot[0] / (counts[0] + 1e-12)
    loss2 = -tot[1] / (counts[1] + 1e-12)
    return np.float32(loss1 + loss2)


# revision 25
# speedup vs baseline: 1.0370x; 1.0370x over previous
"""Trainium2 Bass kernel for nn_DirectionalContrastiveLoss (8-core SPMD).

Strategy (per spec sharding hint): shard the anchor/row dimension across the 8
cores, replicate the host-assembled memory bank, compute each core's score
block locally, and combine masked sums / counts on the host.

V2 layout (vs the earlier selector-based version):
- The label mask is folded INTO the matmul: the contraction gains a third
  K=32 one-hot pass (sqrt(P)*onehot(mem_label_row) vs -sqrt(P)*onehot(
  anchor_label_col), P=1000) so killed columns leave the PE at score-P and
  vanish in the exp.  No kill-range re-reads, no selector matrix.
- Fills are exact 1024-column PSUM tiles (2 banks x 4 slots) so the DVE max
  and ACT exp passes pay the per-instruction overhead only 8x per chain.
- pos (positive logits) and the pos-masks are host-precomputed inputs.
- Per-chain flash combine: GPSIMD min-tree for the running max, one fused
  DVE tensor_tensor_reduce for sum(sparts*edel).
"""
import numpy as np
import ml_dtypes

import bass_rust
import concourse.bass as bass
import concourse.tile as tile
from concourse import mybir
from concourse.bass_utils import run_bass_kernel_spmd
from concourse.vector_clock import ScopedClock

BF16 = ml_dtypes.bfloat16
N_CORES = 8
TEMP = 0.1
POS_THRESH = 0.7
EPS = 1e-8
N = 8000          # anchors (== memory slots)
C = 256           # feature channels
NLAB = 21         # pseudo-label values 0..20
RPC = 1024        # rows per core per direction (padded)
NT = RPC // 128   # row tiles per direction
FILL = 1024       # PSUM fill width (2 banks of fp32)
MM_CHUNK = 512    # matmul free-dim chunk (1 PSUM bank)
SQP = 31.625      # sqrt(penalty); exactly representable in bf16; P ~ 1000

FILLS = [(s, min(FILL, N - s)) for s in range(0, N, FILL)]
NF = len(FILLS)

LAST_RESULTS = None  # BassKernelResults of the most recent kernel() call

# ---------------------------------------------------------------------------
# walrus in this toolchain rejects >1 sync wait per instruction; spread the
# TileContext tail-drain waits over single-wait sync NOPs.
_N_SPILL_NOPS = 64


def _patched_drain_and_barrier(self, tick_clock, wait_clock):
    nops = [self.nc.sync.nop(nofuse=True, hint=f"drainwait{i}")
            for i in range(_N_SPILL_NOPS)]
    drain_inst = self.nc.sync.drain()
    wait_clock.add_sem_waits(drain_inst.ins,
                             ScopedClock({None: tick_clock.global_clock}))
    si = drain_inst.ins.sync_info
    waits = list(si.on_wait) if si is not None else []
    if waits:
        assert len(waits) <= _N_SPILL_NOPS
        for i, w in enumerate(waits):
            nops[i].ins.sync_info = bass_rust.SyncInfo(on_wait=[w], on_update=[])
        drain_inst.ins.sync_info = bass_rust.SyncInfo(
            on_wait=[], on_update=list(si.on_update))
    self.nc.all_engine_barrier()
    popped = self.nc._tile_sem_poison_stack.pop()
    assert popped is self._sem_poison
    self.nc.clear_and_free_semaphores(list(self.sems.allocated().values()))


tile.TileContext._drain_and_barrier = _patched_drain_and_barrier

# Same walrus limitation for regular scheduled instructions: split any
# multi-wait instruction into single-wait same-engine NOPs + the instruction
# keeping its last wait (sequential waits on one engine are equivalent).
_orig_lower_ordered = tile.TileContext._lower_ordered_insts


def _split_multiwait_lower(self, ordered):
    for bb, insts in ordered.items():
        out = []
        for inst in insts:
            si = inst.sync_info
            waits = list(si.on_wait) if si is not None else []
            if len(waits) > 1:
                for w in waits[:-1]:
                    out.append(mybir.InstNoOp(
                        name=self.nc.get_next_instruction_name(),
                        sync_info=mybir.SyncInfo(on_wait=[w], on_update=[]),
                        engine=inst.engine,
                        bass_nofuse=True,
                        text_hint="waitsplit",
                    ))
                inst.sync_info = mybir.SyncInfo(
                    on_wait=[waits[-1]], on_update=list(si.on_update))
            out.append(inst)
        ordered[bb] = out
    return _orig_lower_ordered(self, ordered)


tile.TileContext._lower_ordered_insts = _split_multiwait_lower


# ---------------------------------------------------------------------------
def _build_program(emit):
    """Build the SPMD Bass program (shared by all 8 cores).

    emit: set of (d, t, col_start) 512-col chunks that need the penalty
    matmul pass (a label group killed by some row of tile t intersects it).
    """
    nc = bass.Bass("TRN2", target_bir_lowering=False, debug=False,
                   num_devices=N_CORES)
    f32, bf16 = mybir.dt.float32, mybir.dt.bfloat16
    AX = mybir.AxisListType.X
    OP = mybir.AluOpType
    ACT = mybir.ActivationFunctionType

    d_bank = [nc.dram_tensor(f"bank{d}", [2, 128, N], bf16,
                             kind="ExternalInput").ap() for d in range(2)]
    d_fT = [nc.dram_tensor(f"f{d}T", [2, 128, RPC], bf16,
                           kind="ExternalInput").ap() for d in range(2)]
    d_colpen = [nc.dram_tensor(f"colpen{d}", [32, N], bf16,
                               kind="ExternalInput").ap() for d in range(2)]
    d_rowpen = nc.dram_tensor("rowpen", [32, RPC], bf16,
                              kind="ExternalInput").ap()
    d_negpos = nc.dram_tensor("negpos", [128, NT], f32,
                              kind="ExternalInput").ap()
    d_pm = [nc.dram_tensor(f"pm{d}", [128, NT], f32,
                           kind="ExternalInput").ap() for d in range(2)]
    d_out = nc.dram_tensor("partials", [128, 2], f32, kind="ExternalOutput").ap()

    with tile.TileContext(nc) as tc:
        import contextlib
        with contextlib.ExitStack() as ctx:
            singles = ctx.enter_context(tc.tile_pool(name="singles", bufs=1))
            psum = ctx.enter_context(tc.tile_pool(name="psum", bufs=4, space="PSUM"))
            stats = ctx.enter_context(tc.tile_pool(name="stats", bufs=12))
            scratch = ctx.enter_context(tc.tile_pool(name="scratch", bufs=3))

            # ---- resident inputs ----
            bank = [[singles.tile([128, N], bf16, tag=f"bank{d}k{k}", name=f"bank{d}k{k}")
                     for k in range(2)] for d in range(2)]
            fT = [[singles.tile([128, RPC], bf16, tag=f"fT{d}k{k}", name=f"fT{d}k{k}")
                   for k in range(2)] for d in range(2)]
            colpen = [singles.tile([32, N], bf16, tag=f"colpen{d}", name=f"colpen{d}")
                      for d in range(2)]
            rowpen = singles.tile([32, RPC], bf16, tag="rowpen", name="rowpen")
            negpos = singles.tile([128, NT], f32, tag="negpos", name="negpos")
            pm = [singles.tile([128, NT], f32, tag=f"pm{d}", name=f"pm{d}") for d in range(2)]
            mcol = [singles.tile([128, NT], f32, tag=f"mcol{d}", name=f"mcol{d}") for d in range(2)]
            scol = [singles.tile([128, NT], f32, tag=f"scol{d}", name=f"scol{d}") for d in range(2)]
            numc = [singles.tile([128, NT], f32, tag=f"numc{d}", name=f"numc{d}") for d in range(2)]

            # Each dma_start is a ~650ns DIRECT2D instruction on the serial
            # Sync queue, so the descriptor COUNT (not transfer size) gates
            # the data feed.  Keep only chain 0's first-fill pieces plus a
            # couple of follow-ups on the sync queue, and push the bulk
            # transfers onto the idle GpSimd queue (SWDGE).
            nc.sync.dma_start(out=fT[0][0], in_=d_fT[0][0])
            nc.sync.dma_start(out=bank[0][0][:, 0:1024], in_=d_bank[0][0][:, 0:1024])
            nc.sync.dma_start(out=fT[0][1], in_=d_fT[0][1])
            nc.sync.dma_start(out=bank[0][1][:, 0:1024], in_=d_bank[0][1][:, 0:1024])
            nc.sync.dma_start(out=rowpen, in_=d_rowpen)
            nc.sync.dma_start(out=colpen[0], in_=d_colpen[0])
            nc.sync.dma_start(out=bank[0][0][:, 1024:3072], in_=d_bank[0][0][:, 1024:3072])
            nc.sync.dma_start(out=bank[0][1][:, 1024:3072], in_=d_bank[0][1][:, 1024:3072])
            nc.sync.dma_start(out=negpos, in_=d_negpos)
            for cst in (3072, 5120, 7168):
                ce = min(cst + 2048, N)
                for k in range(2):
                    nc.sync.dma_start(out=bank[0][k][:, cst:ce],
                                      in_=d_bank[0][k][:, cst:ce])
            for d in range(2):
                nc.sync.dma_start(out=pm[d], in_=d_pm[d])
            for k in range(2):
                nc.sync.dma_start(out=fT[1][k], in_=d_fT[1][k])
            nc.sync.dma_start(out=colpen[1], in_=d_colpen[1])
            for cst in (0, 2048, 4096, 6144):
                ce = min(cst + 2048, N)
                for k in range(2):
                    nc.sync.dma_start(out=bank[1][k][:, cst:ce],
                                      in_=d_bank[1][k][:, cst:ce])

            # Preload the Exp/Ln activation table off the critical path: a
            # dummy exp on a memset tile runs as soon as the engines start.
            warm = singles.tile([128, 1], f32, tag="warm", name="warm")
            nc.gpsimd.memset(warm, 0.0)
            warm2 = singles.tile([128, 1], f32, tag="warm2", name="warm2")
            nc.scalar.activation(out=warm2, in_=warm, func=ACT.Exp,
                                 bias=warm[:, 0:1], scale=1.0)

            # Warm the PE (HAM un-throttles after ~3.4us of sustained MMs):
            # dummy matmuls on never-written SBUF garbage, into the first
            # PSUM pool slot; chain 0's fill 0 overwrites it (start=True).
            wsrc = singles.tile([128, 640], bf16, tag="wsrc", name="wsrc")
            nc.gpsimd.memset(wsrc, 0.0)
            wps = psum.tile([128, FILL], f32, tag="ps", name="wps")
            for i in range(22):
                nc.tensor.matmul(wps[:, 0:512], wsrc[:, 0:128],
                                 wsrc[:, 128:640], start=True, stop=True)

            # ---- main loop: 16 chains (2 directions x 8 row tiles) ----
            def chain(d, t):
                    lhs = [fT[d][k][:, t * 128:(t + 1) * 128] for k in range(2)]
                    rp = rowpen[:, t * 128:(t + 1) * 128]
                    # slot NF carries the pos column: negm gets -pos (so the
                    # min-reduce yields -max(fill maxes, pos)) and sparts gets
                    # 1.0 (so sum(sparts*edel) includes exp(pos - m-hat)).
                    negm = stats.tile([128, NF + 1], f32, tag="negm", name="negm")
                    sparts = stats.tile([128, NF + 1], f32, tag="sparts", name="sparts")
                    nc.gpsimd.tensor_copy(out=negm[:, NF:NF + 1],
                                          in_=negpos[:, t:t + 1])
                    nc.gpsimd.memset(sparts[:, NF:NF + 1], 1.0)
                    for fi, (cst, w) in enumerate(FILLS):
                        ps = psum.tile([128, FILL], f32, tag="ps", name="ps")
                        chunks = []
                        off = 0
                        while off < w:
                            cw = min(MM_CHUNK, w - off)
                            chunks.append((off, cw, (d, t, cst + off) in emit))
                            off += cw
                        # k-major order: one LDWEIGHTS per contraction pass
                        # per fill instead of one per chunk.
                        for k in range(2):
                            for (off, cw, pen) in chunks:
                                nc.tensor.matmul(
                                    ps[:, off:off + cw], lhs[k],
                                    bank[d][k][:, cst + off:cst + off + cw],
                                    start=(k == 0),
                                    stop=(k == 1 and not pen))
                        for (off, cw, pen) in chunks:
                            if pen:
                                nc.tensor.matmul(
                                    ps[:, off:off + cw], rp,
                                    colpen[d][:, cst + off:cst + off + cw],
                                    start=False, stop=True)
                        nc.vector.reduce_max(out=negm[:, fi:fi + 1], in_=ps[:, :w],
                                             axis=AX, negate=True)
                        eo = scratch.tile([128, FILL], bf16, tag="eo", name="eo")
                        nc.scalar.activation(
                            out=eo[:, :w], in_=ps[:, :w], func=ACT.Exp,
                            bias=negm[:, fi:fi + 1], scale=1.0,
                            accum_out=sparts[:, fi:fi + 1])
                        yield
                    # flash combine: m-hat = max(fill maxes, pos), via one DVE
                    # min-reduce in the negated domain (slot NF holds -pos).
                    nc.vector.tensor_reduce(out=mcol[d][:, t:t + 1], in_=negm,
                                            axis=AX, op=OP.min)
                    # edel_f = exp(m_f - m-hat); slot NF = exp(pos - m-hat)
                    # which is exactly the logit numerator.
                    edel = stats.tile([128, NF + 1], f32, tag="edel", name="edel")
                    nc.scalar.activation(out=edel, in_=negm, func=ACT.Exp,
                                         bias=mcol[d][:, t:t + 1], scale=-1.0)
                    nc.gpsimd.tensor_copy(out=numc[d][:, t:t + 1],
                                          in_=edel[:, NF:NF + 1])
                    # scol = sum_f sparts_f*edel_f + exp(pos - m-hat): the
                    # full softmax denominator including the pos column, via
                    # one fused multiply+accumulate DVE op.
                    sprod = stats.tile([128, NF + 1], f32, tag="sprod", name="sprod")
                    nc.vector.scalar_tensor_tensor(
                        out=sprod, in0=sparts, scalar=1.0, in1=edel,
                        op0=OP.mult, op1=OP.mult,
                        accum_out=scol[d][:, t:t + 1])

            # ---- final math for one direction, batched over row tiles ----
            outt = singles.tile([128, 2], f32, tag="outt", name="outt")

            def final(d):
                den = stats.tile([128, NT], f32, tag="den", name="den")
                nc.vector.tensor_single_scalar(out=den, in_=scol[d], scalar=EPS,
                                               op=OP.add)
                rec = stats.tile([128, NT], f32, tag="rec", name="rec")
                nc.vector.reciprocal(out=rec, in_=den)
                lg = stats.tile([128, NT], f32, tag="lg", name="lg")
                nc.vector.tensor_tensor(out=lg, in0=numc[d], in1=rec, op=OP.mult)
                lga = stats.tile([128, NT], f32, tag="lga", name="lga")
                nc.vector.tensor_single_scalar(out=lga, in_=lg, scalar=EPS, op=OP.add)
                ll = stats.tile([128, NT], f32, tag="ll", name="ll")
                nc.scalar.activation(out=ll, in_=lga, func=ACT.Ln)
                lm = stats.tile([128, NT], f32, tag="lm", name="lm")
                nc.vector.tensor_tensor(out=lm, in0=ll, in1=pm[d], op=OP.mult)
                nc.vector.reduce_sum(out=outt[:, d:d + 1], in_=lm, axis=AX)

            # Drive: interleave the first two chains fill-by-fill so chain 0's
            # DMA-paced head is hidden behind chain 1's compute; each
            # direction's final math is emitted as soon as its chains finish.
            for d in range(2):
                if d == 0:
                    alive = [chain(0, 0), chain(0, 1)]
                    while alive:
                        for g in list(alive):
                            try:
                                next(g)
                            except StopIteration:
                                alive.remove(g)
                    rest = range(2, NT)
                else:
                    rest = range(NT)
                for t in rest:
                    for _ in chain(d, t):
                        pass
                final(d)
            nc.sync.dma_start(out=d_out, in_=outt)

    return nc


# ---------------------------------------------------------------------------
def kernel(output_feat1, output_feat2, pseudo_label1, pseudo_label2,
           pseudo_logits1, pseudo_logits2, output_ul1, output_ul2,
           selected_idx1, selected_idx2):
    f1 = np.ascontiguousarray(np.asarray(output_feat1, dtype=np.float32))
    f2 = np.ascontiguousarray(np.asarray(output_feat2, dtype=np.float32))
    pl1 = np.asarray(pseudo_label1).astype(np.int64)
    pl2 = np.asarray(pseudo_label2).astype(np.int64)
    pg1 = np.asarray(pseudo_logits1, dtype=np.float32)
    pg2 = np.asarray(pseudo_logits2, dtype=np.float32)
    ul1 = np.asarray(output_ul1, dtype=np.float32)
    ul2 = np.asarray(output_ul2, dtype=np.float32)
    idx1 = np.asarray(selected_idx1).astype(np.int64)
    idx2 = np.asarray(selected_idx2).astype(np.int64)

    b, c, h, w = ul1.shape
    ul1f = ul1.transpose(0, 2, 3, 1).reshape(-1, c)
    ul2f = ul2.transpose(0, 2, 3, 1).reshape(-1, c)
    bank_vals = np.concatenate([ul1f[idx1], ul2f[idx2]], axis=0)   # [N, C]
    ml = np.concatenate([pl1[idx1], pl2[idx2]], axis=0)            # [N]

    # --- column layout per direction (transposed-bug mask: col j kills rows
    # whose memory label equals pl_d[j]); columns sorted by pl_d.
    banks, colpens, colranges = [], [], []
    for pl in (pl1, pl2):
        order = np.argsort(pl, kind="stable")
        sizes = np.bincount(pl, minlength=NLAB)
        starts = np.concatenate([[0], np.cumsum(sizes)])
        colranges.append([(int(starts[v]), int(starts[v + 1])) for v in range(NLAB)])
        bT = np.ascontiguousarray(bank_vals[order].T.astype(BF16))  # [C, N]
        banks.append(bT.reshape(2, 128, N))
        cp = np.zeros((32, N), dtype=np.float32)
        cp[pl[order], np.arange(N)] = -SQP
        colpens.append(cp.astype(BF16))

    # --- row layout: label-sorted (by memory label) with fixed per-core quotas
    nv = np.bincount(ml, minlength=NLAB)
    qv = (nv + N_CORES - 1) // N_CORES
    assert qv.sum() <= RPC
    row_segs = []
    p = 0
    for v in range(NLAB):
        if qv[v] > 0:
            row_segs.append((p, p + int(qv[v]), v))
            p += int(qv[v])

    rows_sorted = np.argsort(ml, kind="stable")
    starts_ml = np.concatenate([[0], np.cumsum(nv)])
    perms = np.full((N_CORES, RPC), -1, dtype=np.int64)
    for v in range(NLAB):
        seg = next(s for s in row_segs if s[2] == v)
        rows_v = rows_sorted[starts_ml[v]:starts_ml[v + 1]]
        for core in range(N_CORES):
            chunk = rows_v[core * qv[v]:(core + 1) * qv[v]]
            perms[core, seg[0]:seg[0] + len(chunk)] = chunk

    # rowpen: [32, RPC] one-hot of each padded row's memory label, sqrt(P).
    rowpen = np.zeros((32, RPC), dtype=np.float32)
    rowlab = np.full(RPC, -1, dtype=np.int64)
    for (s0, s1, v) in row_segs:
        rowlab[s0:s1] = v
        rowpen[v, s0:s1] = SQP
    rowpen = rowpen.astype(BF16)

    # penalty-chunk emit set: (d, t, chunk_start) where a label group killed
    # by some row of tile t intersects the 512-col chunk.
    emit = set()
    for d in range(2):
        for t in range(NT):
            labs = set(rowlab[t * 128:(t + 1) * 128].tolist()) - {-1}
            for (cst, wf) in FILLS:
                off = 0
                while off < wf:
                    cw = min(MM_CHUNK, wf - off)
                    c0, c1 = cst + off, cst + off + cw
                    for v in labs:
                        g0, g1 = colranges[d][v]
                        if g0 < c1 and c0 < g1:
                            emit.add((d, t, c0))
                            break
                    off += cw

    # --- per-core input maps
    def gather_rows(x, perm):
        out = np.zeros((RPC,) + x.shape[1:], dtype=x.dtype)
        msk = perm >= 0
        out[msk] = x[perm[msk]]
        return out

    # host-side row stats: pos (identical for both directions), pos masks
    pos_full = (f1 * f2).sum(axis=1) * (1.0 / TEMP)                # [N]
    pm_full = [((pg2 > POS_THRESH) & (pg1 < pg2)).astype(np.float32),
               ((pg1 > POS_THRESH) & (pg2 < pg1)).astype(np.float32)]
    counts = [float(pm_full[0].sum()), float(pm_full[1].sum())]

    def tcol(x):  # [RPC] -> [128, NT] (tile-major columns)
        return np.ascontiguousarray(x.reshape(NT, 128).T)

    in_maps = []
    for core in range(N_CORES):
        perm = perms[core]
        fc = [gather_rows(f1, perm), gather_rows(f2, perm)]
        posc = gather_rows(pos_full, perm)
        m = {"rowpen": rowpen, "negpos": tcol(-posc)}
        for d in range(2):
            m[f"bank{d}"] = banks[d]
            m[f"colpen{d}"] = colpens[d]
            fT = np.ascontiguousarray((fc[d].T * (1.0 / TEMP)).astype(BF16))
            m[f"f{d}T"] = fT.reshape(2, 128, RPC)
            m[f"pm{d}"] = tcol(gather_rows(pm_full[d], perm))
        in_maps.append(m)

    nc = _build_program(emit)
    res = run_bass_kernel_spmd(nc, in_maps, list(range(N_CORES)))
    global LAST_RESULTS
    LAST_RESULTS = res

    tot = np.zeros(2, dtype=np.float64)
    for core in range(N_CORES):
        tot += res.results[core]["partials"].astype(np.float64).sum(axis=0)
    loss1 = -tot[0] / (counts[0] + 1e-12)
    loss2 = -tot[1] / (counts[1] + 1e-12)
    return np.float32(loss1 + loss2)


# revision 26
# speedup vs baseline: 1.0372x; 1.0002x over previous
"""Trainium2 Bass kernel for nn_DirectionalContrastiveLoss (8-core SPMD).

Strategy (per spec sharding hint): shard the anchor/row dimension across the 8
cores, replicate the host-assembled memory bank, compute each core's score
block locally, and combine masked sums / counts on the host.

Kernel design:
- The label mask is folded INTO the matmul: the contraction gains a third
  K=32 one-hot pass (sqrt(P)*onehot(mem_label_row) vs -sqrt(P)*onehot(
  anchor_label_col), P~1000) so killed columns leave the PE at score-P and
  vanish in the exp.  No kill-range re-reads, no selector matrix.
- Fills are exact 1024-column PSUM tiles (2 banks x 4 slots, the deepest
  pipeline PSUM allows); per fill: PE matmul (k-major, 2 LDWEIGHTS) ->
  DVE reduce_max (the critical engine, ~96% busy) -> ACT exp with
  accum_out.  Steady state has all three engines within ~4% of each other.
- pos (positive logits) and the pos-masks are host-precomputed inputs; the
  extra slot NF of negm/sparts carries -pos/1.0 so one DVE min-reduce
  yields m-hat including pos and the same ACT edel instruction emits the
  logit numerator exp(pos - m-hat) for free.
- DMA staging: each dma_start costs ~650ns of issue time on the serial
  sync queue, so transfers are few and <=0.5MB (>0.5MB DIRECT2Ds crashed
  the exec unit), ordered so chain 0's fills stream in just-in-time; the
  first two chains are emitted interleaved to ride out the DMA-paced head.
- Warmup: a dummy exp preloads the ACT table and 22 garbage matmuls warm
  the PE HAM clock gate before the first real fill arrives.
"""
import numpy as np
import ml_dtypes

import bass_rust
import concourse.bass as bass
import concourse.tile as tile
from concourse import mybir
from concourse.bass_utils import run_bass_kernel_spmd
from concourse.vector_clock import ScopedClock

BF16 = ml_dtypes.bfloat16
N_CORES = 8
TEMP = 0.1
POS_THRESH = 0.7
EPS = 1e-8
N = 8000          # anchors (== memory slots)
C = 256           # feature channels
NLAB = 21         # pseudo-label values 0..20
RPC = 1024        # rows per core per direction (padded)
NT = RPC // 128   # row tiles per direction
FILL = 1024       # PSUM fill width (2 banks of fp32)
MM_CHUNK = 512    # matmul free-dim chunk (1 PSUM bank)
SQP = 31.625      # sqrt(penalty); exactly representable in bf16; P ~ 1000

FILLS = [(s, min(FILL, N - s)) for s in range(0, N, FILL)]
NF = len(FILLS)

LAST_RESULTS = None  # BassKernelResults of the most recent kernel() call

# ---------------------------------------------------------------------------
# walrus in this toolchain rejects >1 sync wait per instruction; spread the
# TileContext tail-drain waits over single-wait sync NOPs.
_N_SPILL_NOPS = 64


def _patched_drain_and_barrier(self, tick_clock, wait_clock):
    nops = [self.nc.sync.nop(nofuse=True, hint=f"drainwait{i}")
            for i in range(_N_SPILL_NOPS)]
    drain_inst = self.nc.sync.drain()
    wait_clock.add_sem_waits(drain_inst.ins,
                             ScopedClock({None: tick_clock.global_clock}))
    si = drain_inst.ins.sync_info
    waits = list(si.on_wait) if si is not None else []
    if waits:
        assert len(waits) <= _N_SPILL_NOPS
        for i, w in enumerate(waits):
            nops[i].ins.sync_info = bass_rust.SyncInfo(on_wait=[w], on_update=[])
        drain_inst.ins.sync_info = bass_rust.SyncInfo(
            on_wait=[], on_update=list(si.on_update))
    self.nc.all_engine_barrier()
    popped = self.nc._tile_sem_poison_stack.pop()
    assert popped is self._sem_poison
    self.nc.clear_and_free_semaphores(list(self.sems.allocated().values()))


tile.TileContext._drain_and_barrier = _patched_drain_and_barrier

# Same walrus limitation for regular scheduled instructions: split any
# multi-wait instruction into single-wait same-engine NOPs + the instruction
# keeping its last wait (sequential waits on one engine are equivalent).
_orig_lower_ordered = tile.TileContext._lower_ordered_insts


def _split_multiwait_lower(self, ordered):
    for bb, insts in ordered.items():
        out = []
        for inst in insts:
            si = inst.sync_info
            waits = list(si.on_wait) if si is not None else []
            if len(waits) > 1:
                for w in waits[:-1]:
                    out.append(mybir.InstNoOp(
                        name=self.nc.get_next_instruction_name(),
                        sync_info=mybir.SyncInfo(on_wait=[w], on_update=[]),
                        engine=inst.engine,
                        bass_nofuse=True,
                        text_hint="waitsplit",
                    ))
                inst.sync_info = mybir.SyncInfo(
                    on_wait=[waits[-1]], on_update=list(si.on_update))
            out.append(inst)
        ordered[bb] = out
    return _orig_lower_ordered(self, ordered)


tile.TileContext._lower_ordered_insts = _split_multiwait_lower


# ---------------------------------------------------------------------------
def _build_program(emit):
    """Build the SPMD Bass program (shared by all 8 cores).

    emit: set of (d, t, col_start) 512-col chunks that need the penalty
    matmul pass (a label group killed by some row of tile t intersects it).
    """
    nc = bass.Bass("TRN2", target_bir_lowering=False, debug=False,
                   num_devices=N_CORES)
    f32, bf16 = mybir.dt.float32, mybir.dt.bfloat16
    AX = mybir.AxisListType.X
    OP = mybir.AluOpType
    ACT = mybir.ActivationFunctionType

    d_bank = [nc.dram_tensor(f"bank{d}", [2, 128, N], bf16,
                             kind="ExternalInput").ap() for d in range(2)]
    d_fT = [nc.dram_tensor(f"f{d}T", [2, 128, RPC], bf16,
                           kind="ExternalInput").ap() for d in range(2)]
    d_colpen = [nc.dram_tensor(f"colpen{d}", [32, N], bf16,
                               kind="ExternalInput").ap() for d in range(2)]
    d_rowpen = nc.dram_tensor("rowpen", [32, RPC], bf16,
                              kind="ExternalInput").ap()
    d_negpos = nc.dram_tensor("negpos", [128, NT], f32,
                              kind="ExternalInput").ap()
    d_pm = [nc.dram_tensor(f"pm{d}", [128, NT], f32,
                           kind="ExternalInput").ap() for d in range(2)]
    d_out = nc.dram_tensor("partials", [128, 2], f32, kind="ExternalOutput").ap()

    with tile.TileContext(nc) as tc:
        import contextlib
        with contextlib.ExitStack() as ctx:
            singles = ctx.enter_context(tc.tile_pool(name="singles", bufs=1))
            psum = ctx.enter_context(tc.tile_pool(name="psum", bufs=4, space="PSUM"))
            stats = ctx.enter_context(tc.tile_pool(name="stats", bufs=12))
            scratch = ctx.enter_context(tc.tile_pool(name="scratch", bufs=3))

            # ---- resident inputs ----
            bank = [[singles.tile([128, N], bf16, tag=f"bank{d}k{k}", name=f"bank{d}k{k}")
                     for k in range(2)] for d in range(2)]
            fT = [[singles.tile([128, RPC], bf16, tag=f"fT{d}k{k}", name=f"fT{d}k{k}")
                   for k in range(2)] for d in range(2)]
            colpen = [singles.tile([32, N], bf16, tag=f"colpen{d}", name=f"colpen{d}")
                      for d in range(2)]
            rowpen = singles.tile([32, RPC], bf16, tag="rowpen", name="rowpen")
            negpos = singles.tile([128, NT], f32, tag="negpos", name="negpos")
            pm = [singles.tile([128, NT], f32, tag=f"pm{d}", name=f"pm{d}") for d in range(2)]
            mcol = [singles.tile([128, NT], f32, tag=f"mcol{d}", name=f"mcol{d}") for d in range(2)]
            scol = [singles.tile([128, NT], f32, tag=f"scol{d}", name=f"scol{d}") for d in range(2)]
            numc = [singles.tile([128, NT], f32, tag=f"numc{d}", name=f"numc{d}") for d in range(2)]

            # Each dma_start is a ~650ns DIRECT2D instruction on the serial
            # Sync queue, so the descriptor COUNT (not transfer size) gates
            # the data feed.  Keep only chain 0's first-fill pieces plus a
            # couple of follow-ups on the sync queue, and push the bulk
            # transfers onto the idle GpSimd queue (SWDGE).
            nc.sync.dma_start(out=fT[0][0], in_=d_fT[0][0])
            nc.sync.dma_start(out=bank[0][0][:, 0:1024], in_=d_bank[0][0][:, 0:1024])
            nc.sync.dma_start(out=fT[0][1], in_=d_fT[0][1])
            nc.sync.dma_start(out=bank[0][1][:, 0:1024], in_=d_bank[0][1][:, 0:1024])
            nc.sync.dma_start(out=rowpen, in_=d_rowpen)
            nc.sync.dma_start(out=colpen[0], in_=d_colpen[0])
            nc.sync.dma_start(out=bank[0][0][:, 1024:3072], in_=d_bank[0][0][:, 1024:3072])
            nc.sync.dma_start(out=bank[0][1][:, 1024:3072], in_=d_bank[0][1][:, 1024:3072])
            nc.sync.dma_start(out=negpos, in_=d_negpos)
            for cst in (3072, 5120, 7168):
                ce = min(cst + 2048, N)
                for k in range(2):
                    nc.sync.dma_start(out=bank[0][k][:, cst:ce],
                                      in_=d_bank[0][k][:, cst:ce])
            for d in range(2):
                nc.sync.dma_start(out=pm[d], in_=d_pm[d])
            for k in range(2):
                nc.sync.dma_start(out=fT[1][k], in_=d_fT[1][k])
            nc.sync.dma_start(out=colpen[1], in_=d_colpen[1])
            for cst in (0, 2048, 4096, 6144):
                ce = min(cst + 2048, N)
                for k in range(2):
                    nc.sync.dma_start(out=bank[1][k][:, cst:ce],
                                      in_=d_bank[1][k][:, cst:ce])

            # Preload the Exp/Ln activation table off the critical path: a
            # dummy exp on a memset tile runs as soon as the engines start.
            warm = singles.tile([128, 1], f32, tag="warm", name="warm")
            nc.gpsimd.memset(warm, 0.0)
            warm2 = singles.tile([128, 1], f32, tag="warm2", name="warm2")
            nc.scalar.activation(out=warm2, in_=warm, func=ACT.Exp,
                                 bias=warm[:, 0:1], scale=1.0)

            # Warm the PE (HAM un-throttles after ~3.4us of sustained MMs):
            # dummy matmuls on never-written SBUF garbage, into the first
            # PSUM pool slot; chain 0's fill 0 overwrites it (start=True).
            wsrc = singles.tile([128, 640], bf16, tag="wsrc", name="wsrc")
            nc.gpsimd.memset(wsrc, 0.0)
            wps = psum.tile([128, FILL], f32, tag="ps", name="wps")
            for i in range(22):
                nc.tensor.matmul(wps[:, 0:512], wsrc[:, 0:128],
                                 wsrc[:, 128:640], start=True, stop=True)

            # ---- main loop: 16 chains (2 directions x 8 row tiles) ----
            def chain(d, t):
                    lhs = [fT[d][k][:, t * 128:(t + 1) * 128] for k in range(2)]
                    rp = rowpen[:, t * 128:(t + 1) * 128]
                    # slot NF carries the pos column: negm gets -pos (so the
                    # min-reduce yields -max(fill maxes, pos)) and sparts gets
                    # 1.0 (so sum(sparts*edel) includes exp(pos - m-hat)).
                    negm = stats.tile([128, NF + 1], f32, tag="negm", name="negm")
                    sparts = stats.tile([128, NF + 1], f32, tag="sparts", name="sparts")
                    nc.gpsimd.tensor_copy(out=negm[:, NF:NF + 1],
                                          in_=negpos[:, t:t + 1])
                    nc.gpsimd.memset(sparts[:, NF:NF + 1], 1.0)
                    for fi, (cst, w) in enumerate(FILLS):
                        ps = psum.tile([128, FILL], f32, tag="ps", name="ps")
                        chunks = []
                        off = 0
                        while off < w:
                            cw = min(MM_CHUNK, w - off)
                            chunks.append((off, cw, (d, t, cst + off) in emit))
                            off += cw
                        # k-major order: one LDWEIGHTS per contraction pass
                        # per fill instead of one per chunk.
                        for k in range(2):
                            for (off, cw, pen) in chunks:
                                nc.tensor.matmul(
                                    ps[:, off:off + cw], lhs[k],
                                    bank[d][k][:, cst + off:cst + off + cw],
                                    start=(k == 0),
                                    stop=(k == 1 and not pen))
                        for (off, cw, pen) in chunks:
                            if pen:
                                nc.tensor.matmul(
                                    ps[:, off:off + cw], rp,
                                    colpen[d][:, cst + off:cst + off + cw],
                                    start=False, stop=True)
                        nc.vector.reduce_max(out=negm[:, fi:fi + 1], in_=ps[:, :w],
                                             axis=AX, negate=True)
                        eo = scratch.tile([128, FILL], bf16, tag="eo", name="eo")
                        nc.scalar.activation(
                            out=eo[:, :w], in_=ps[:, :w], func=ACT.Exp,
                            bias=negm[:, fi:fi + 1], scale=1.0,
                            accum_out=sparts[:, fi:fi + 1])
                        yield
                    # flash combine: m-hat = max(fill maxes, pos), via one DVE
                    # min-reduce in the negated domain (slot NF holds -pos).
                    nc.vector.tensor_reduce(out=mcol[d][:, t:t + 1], in_=negm,
                                            axis=AX, op=OP.min)
                    # edel_f = exp(m_f - m-hat); slot NF = exp(pos - m-hat)
                    # which is exactly the logit numerator.
                    edel = stats.tile([128, NF + 1], f32, tag="edel", name="edel")
                    nc.scalar.activation(out=edel, in_=negm, func=ACT.Exp,
                                         bias=mcol[d][:, t:t + 1], scale=-1.0)
                    nc.gpsimd.tensor_copy(out=numc[d][:, t:t + 1],
                                          in_=edel[:, NF:NF + 1])
                    # scol = sum_f sparts_f*edel_f + exp(pos - m-hat): the
                    # full softmax denominator including the pos column, via
                    # one fused multiply+accumulate DVE op.
                    sprod = stats.tile([128, NF + 1], f32, tag="sprod", name="sprod")
                    nc.vector.scalar_tensor_tensor(
                        out=sprod, in0=sparts, scalar=1.0, in1=edel,
                        op0=OP.mult, op1=OP.mult,
                        accum_out=scol[d][:, t:t + 1])

            # ---- final math for one direction, batched over row tiles ----
            outt = singles.tile([128, 2], f32, tag="outt", name="outt")

            def final(d):
                den = stats.tile([128, NT], f32, tag="den", name="den")
                nc.vector.tensor_single_scalar(out=den, in_=scol[d], scalar=EPS,
                                               op=OP.add)
                rec = stats.tile([128, NT], f32, tag="rec", name="rec")
                nc.vector.reciprocal(out=rec, in_=den)
                lg = stats.tile([128, NT], f32, tag="lg", name="lg")
                nc.vector.tensor_tensor(out=lg, in0=numc[d], in1=rec, op=OP.mult)
                lga = stats.tile([128, NT], f32, tag="lga", name="lga")
                nc.vector.tensor_single_scalar(out=lga, in_=lg, scalar=EPS, op=OP.add)
                ll = stats.tile([128, NT], f32, tag="ll", name="ll")
                nc.scalar.activation(out=ll, in_=lga, func=ACT.Ln)
                lm = stats.tile([128, NT], f32, tag="lm", name="lm")
                nc.vector.tensor_tensor(out=lm, in0=ll, in1=pm[d], op=OP.mult)
                nc.vector.reduce_sum(out=outt[:, d:d + 1], in_=lm, axis=AX)

            # Drive: interleave the first two chains fill-by-fill so chain 0's
            # DMA-paced head is hidden behind chain 1's compute; each
            # direction's final math is emitted as soon as its chains finish.
            for d in range(2):
                if d == 0:
                    alive = [chain(0, 0), chain(0, 1)]
                    while alive:
                        for g in list(alive):
                            try:
                                next(g)
                            except StopIteration:
                                alive.remove(g)
                    rest = range(2, NT)
                else:
                    rest = range(NT)
                for t in rest:
                    for _ in chain(d, t):
                        pass
                final(d)
            nc.sync.dma_start(out=d_out, in_=outt)

    return nc


# ---------------------------------------------------------------------------
def kernel(output_feat1, output_feat2, pseudo_label1, pseudo_label2,
           pseudo_logits1, pseudo_logits2, output_ul1, output_ul2,
           selected_idx1, selected_idx2):
    f1 = np.ascontiguousarray(np.asarray(output_feat1, dtype=np.float32))
    f2 = np.ascontiguousarray(np.asarray(output_feat2, dtype=np.float32))
    pl1 = np.asarray(pseudo_label1).astype(np.int64)
    pl2 = np.asarray(pseudo_label2).astype(np.int64)
    pg1 = np.asarray(pseudo_logits1, dtype=np.float32)
    pg2 = np.asarray(pseudo_logits2, dtype=np.float32)
    ul1 = np.asarray(output_ul1, dtype=np.float32)
    ul2 = np.asarray(output_ul2, dtype=np.float32)
    idx1 = np.asarray(selected_idx1).astype(np.int64)
    idx2 = np.asarray(selected_idx2).astype(np.int64)

    b, c, h, w = ul1.shape
    ul1f = ul1.transpose(0, 2, 3, 1).reshape(-1, c)
    ul2f = ul2.transpose(0, 2, 3, 1).reshape(-1, c)
    bank_vals = np.concatenate([ul1f[idx1], ul2f[idx2]], axis=0)   # [N, C]
    ml = np.concatenate([pl1[idx1], pl2[idx2]], axis=0)            # [N]

    # --- column layout per direction (transposed-bug mask: col j kills rows
    # whose memory label equals pl_d[j]); columns sorted by pl_d.
    banks, colpens, colranges = [], [], []
    for pl in (pl1, pl2):
        order = np.argsort(pl, kind="stable")
        sizes = np.bincount(pl, minlength=NLAB)
        starts = np.concatenate([[0], np.cumsum(sizes)])
        colranges.append([(int(starts[v]), int(starts[v + 1])) for v in range(NLAB)])
        bT = np.ascontiguousarray(bank_vals[order].T.astype(BF16))  # [C, N]
        banks.append(bT.reshape(2, 128, N))
        cp = np.zeros((32, N), dtype=np.float32)
        cp[pl[order], np.arange(N)] = -SQP
        colpens.append(cp.astype(BF16))

    # --- row layout: label-sorted (by memory label) with fixed per-core quotas
    nv = np.bincount(ml, minlength=NLAB)
    qv = (nv + N_CORES - 1) // N_CORES
    assert qv.sum() <= RPC
    row_segs = []
    p = 0
    for v in range(NLAB):
        if qv[v] > 0:
            row_segs.append((p, p + int(qv[v]), v))
            p += int(qv[v])

    rows_sorted = np.argsort(ml, kind="stable")
    starts_ml = np.concatenate([[0], np.cumsum(nv)])
    perms = np.full((N_CORES, RPC), -1, dtype=np.int64)
    for v in range(NLAB):
        seg = next(s for s in row_segs if s[2] == v)
        rows_v = rows_sorted[starts_ml[v]:starts_ml[v + 1]]
        for core in range(N_CORES):
            chunk = rows_v[core * qv[v]:(core + 1) * qv[v]]
            perms[core, seg[0]:seg[0] + len(chunk)] = chunk

    # rowpen: [32, RPC] one-hot of each padded row's memory label, sqrt(P).
    rowpen = np.zeros((32, RPC), dtype=np.float32)
    rowlab = np.full(RPC, -1, dtype=np.int64)
    for (s0, s1, v) in row_segs:
        rowlab[s0:s1] = v
        rowpen[v, s0:s1] = SQP
    rowpen = rowpen.astype(BF16)

    # penalty-chunk emit set: (d, t, chunk_start) where a label group killed
    # by some row of tile t intersects the 512-col chunk.
    emit = set()
    for d in range(2):
        for t in range(NT):
            labs = set(rowlab[t * 128:(t + 1) * 128].tolist()) - {-1}
            for (cst, wf) in FILLS:
                off = 0
                while off < wf:
                    cw = min(MM_CHUNK, wf - off)
                    c0, c1 = cst + off, cst + off + cw
                    for v in labs:
                        g0, g1 = colranges[d][v]
                        if g0 < c1 and c0 < g1:
                            emit.add((d, t, c0))
                            break
                    off += cw

    # --- per-core input maps
    def gather_rows(x, perm):
        out = np.zeros((RPC,) + x.shape[1:], dtype=x.dtype)
        msk = perm >= 0
        out[msk] = x[perm[msk]]
        return out

    # host-side row stats: pos (identical for both directions), pos masks
    pos_full = (f1 * f2).sum(axis=1) * (1.0 / TEMP)                # [N]
    pm_full = [((pg2 > POS_THRESH) & (pg1 < pg2)).astype(np.float32),
               ((pg1 > POS_THRESH) & (pg2 < pg1)).astype(np.float32)]
    counts = [float(pm_full[0].sum()), float(pm_full[1].sum())]

    def tcol(x):  # [RPC] -> [128, NT] (tile-major columns)
        return np.ascontiguousarray(x.reshape(NT, 128).T)

    in_maps = []
    for core in range(N_CORES):
        perm = perms[core]
        fc = [gather_rows(f1, perm), gather_rows(f2, perm)]
        posc = gather_rows(pos_full, perm)
        m = {"rowpen": rowpen, "negpos": tcol(-posc)}
        for d in range(2):
            m[f"bank{d}"] = banks[d]
            m[f"colpen{d}"] = colpens[d]
            fT = np.ascontiguousarray((fc[d].T * (1.0 / TEMP)).astype(BF16))
            m[f"f{d}T"] = fT.reshape(2, 128, RPC)
            m[f"pm{d}"] = tcol(gather_rows(pm_full[d], perm))
        in_maps.append(m)

    nc = _build_program(emit)
    res = run_bass_kernel_spmd(nc, in_maps, list(range(N_CORES)))
    global LAST_RESULTS
    LAST_RESULTS = res

    tot = np.zeros(2, dtype=np.float64)
    for core in range(N_CORES):
        tot += res.results[core]["partials"].astype(np.float64).sum(axis=0)
    loss1 = -tot[0] / (counts[0] + 1e-12)
    loss2 = -tot[1] / (counts[1] + 1e-12)
    return np.float32(loss1 + loss2)
